# revision 3
# baseline (speedup 1.0000x reference)
"""Single-head causal attention (B=8, T=2048, C=1024, H=128) on 8 trn2 cores.

Data-parallel over batch: core b computes attention for batch element b.

v3 changes vs v2:
  - startup DMA: few big posts (Wq | Wkv | one 1MB post per x segment,
    s0 split across both HWDGE rings) -> first real matmul ~5us
  - warmup sized to cover DMA landing (64 x N=128, ends ~5us, warm clock)
  - softmax denominators l OFF the PE: diagonal e-tiles zero-filled, all
    strips full-width, bf16 binary-tree adds on VectorE, one ones-matmul
    per q-range (emitted BEFORE PV so the kernel tail is just PV+copy+DMA)
  - l DMA on sync (HWDGE) instead of gpsimd (SWDGE)

Per-core device algorithm (bf16 matmuls, f32 PSUM accum):
  1. qT/kT/vT segments [128, 512] = W.T @ xT   (8 cc chunks each)
  2. v_nat[kt] [128,128] via XBAR dma transpose of vT segment
  3. per q-range r (512 wide):
       full strips kt<4r:  ST=k.q [128,512]; E=exp(ST/sqrt(C))
       diag strips j=0..3: cols [128j,512) computed; [0,128j) memset 0;
                           triangular mask on cols [128j, 128j+128)
       l[r] = ones.T @ (bf16 tree-sum of E strips)     (tree on DVE)
       outT[r] += v_nat[kt].T @ E[kt]   (PSUM accum over strips)
  4. DMA outT[r] (via DVE copy) and l[r] -> DRAM; host does (outT/l).T
"""

import numpy as np

import concourse.bacc as bacc
import concourse.mybir as mybir
import concourse.tile as tile
from concourse.bass_utils import run_bass_kernel_spmd

B, T, C, H = 8, 2048, 1024, 128
NCORES = 8
QR = 512          # q-range width (one PSUM bank)
NQR = T // QR     # 4 q-ranges
NKT = T // 128    # 16 k-strips
NCC = C // 128    # 8 contraction chunks
SCALE = 1.0 / np.sqrt(C)
NWARM = 64        # warmup matmuls (N=128): ~3.4us cold + ~1.7us warm

F32 = mybir.dt.float32
BF16 = mybir.dt.bfloat16


def _build_program():
    nc = bacc.Bacc("TRN2", target_bir_lowering=False, debug=False,
                   num_devices=NCORES, num_swdge_queues=4)

    # x prepped as [s][128 p][cc][512]; one 1MB post per segment
    x_d = nc.dram_tensor("x", [NQR, 128, NCC, QR], BF16, kind="ExternalInput")
    w_d = nc.dram_tensor("w", [128, 3 * NCC * H], BF16, kind="ExternalInput")
    mask_d = nc.dram_tensor("mask", [128, 128], BF16, kind="ExternalInput")
    ones_d = nc.dram_tensor("ones", [128, 1], BF16, kind="ExternalInput")
    out_d = nc.dram_tensor("out", [H, T], F32, kind="ExternalOutput")
    l_d = nc.dram_tensor("l", [1, T], F32, kind="ExternalOutput")

    with tile.TileContext(nc) as tc:
        with (
            tc.tile_pool(name="consts", bufs=1) as consts,
            tc.tile_pool(name="xt", bufs=NQR) as xt_pool,
            tc.tile_pool(name="qkvT", bufs=1) as qkvT_pool,
            tc.tile_pool(name="vnat", bufs=NQR) as vnat_pool,
            tc.tile_pool(name="e", bufs=32) as e_pool,
            tc.tile_pool(name="ep", bufs=16) as ep_pool,
            tc.tile_pool(name="osmall", bufs=1) as osmall_pool,
            tc.tile_pool(name="mm1k", bufs=2, space="PSUM") as mm1k_pool,
            tc.tile_pool(name="st", bufs=3, space="PSUM") as st_pool,
            tc.tile_pool(name="acc", bufs=1, space="PSUM") as acc_pool,
        ):
            # ---- DMA loads: few big posts, s0 split across both rings ----
            w_sb = consts.tile([128, 3, NCC, H], BF16, tag="w")
            w_ap = w_d.ap().rearrange("p (w cc h) -> p w cc h", w=3, cc=NCC)
            xt = [xt_pool.tile([128, NCC, QR], BF16, tag="xt", name=f"xt{s}")
                  for s in range(NQR)]

            # sync ring: Wq, s0a, s1, s3 / scalar ring: s0b, Wkv, s2
            nc.sync.dma_start(w_sb[:, 0:1], w_ap[:, 0:1])
            nc.scalar.dma_start(xt[0][:, 0:4, :], x_d.ap()[0, :, 0:4, :])
            nc.sync.dma_start(xt[0][:, 4:8, :], x_d.ap()[0, :, 4:8, :])
            nc.scalar.dma_start(w_sb[:, 1:3], w_ap[:, 1:3])
            nc.sync.dma_start(xt[1][:], x_d.ap()[1])
            nc.scalar.dma_start(xt[2][:], x_d.ap()[2])
            nc.sync.dma_start(xt[3][:], x_d.ap()[3])
            mask_sb = consts.tile([128, 128], BF16, tag="mask")
            nc.gpsimd.dma_start(mask_sb[:], mask_d.ap())
            ones_sb = consts.tile([128, 1], BF16, tag="ones")
            nc.gpsimd.dma_start(ones_sb[:], ones_d.ap())

            # ---- PE/ACT warmup while DMAs land -----------------------------
            dummyw = consts.tile([128, 128], BF16, tag="dummyw")
            dummyx = consts.tile([128, 128], BF16, tag="dummyx")
            nc.vector.memset(dummyw[:], 1.0)
            nc.vector.memset(dummyx[:], 0.0)
            warm_ps = mm1k_pool.tile([128, QR], F32, tag="mm1k")
            for _ in range(NWARM):
                nc.tensor.matmul(warm_ps[:, 0:128], dummyw[:], dummyx[:],
                                 start=True, stop=True)
            nc.scalar.activation(
                dummyw[:, 0:1], dummyx[:, 0:1],
                mybir.ActivationFunctionType.Exp)

            # ---- qT/kT/vT segments ----------------------------------------
            qTs = [qkvT_pool.tile([128, QR], BF16, tag=f"qT{s}",
                                  name=f"qT{s}") for s in range(NQR)]
            kTs = [qkvT_pool.tile([128, QR], BF16, tag=f"kT{s}",
                                  name=f"kT{s}") for s in range(NQR)]
            vTs = [qkvT_pool.tile([128, QR], BF16, tag=f"vT{s}",
                                  name=f"vT{s}") for s in range(NQR)]

            def kslice(kt):
                return kTs[kt // 4][:, 128 * (kt % 4):128 * (kt % 4 + 1)]

            def emit_qkv(s):
                for wi, dst in ((0, qTs[s]), (1, kTs[s]), (2, vTs[s])):
                    ps = mm1k_pool.tile([128, QR], F32, tag="mm1k")
                    for cc in range(NCC):
                        nc.tensor.matmul(
                            ps[:],
                            w_sb[:, wi, cc, :],
                            xt[s][:, cc, :],
                            start=(cc == 0), stop=(cc == NCC - 1))
                    nc.vector.tensor_copy(dst[:], ps[:])

            # v natural layout via XBAR dma transpose:
            # vnat[p, j, c] = vT[c, 128j + p]
            vnat = [None] * NQR

            def emit_vtr(seg):
                vt = vnat_pool.tile([128, 4, 128], BF16, tag="vnat",
                                    name=f"vnat{seg}")
                nc.sync.dma_start_transpose(vt[:], vTs[seg][:])
                vnat[seg] = vt

            def vslice(kt):
                return vnat[kt // 4][:, kt % 4, :]

            # ---- attention -------------------------------------------------
            all_e = {}

            def emit_st(r):
                nkt = 4 * r + 4
                es = [None] * nkt
                # diagonal strips first so exp+mask clear early
                for kt in list(range(4 * r, nkt)) + list(range(4 * r)):
                    j = kt - 4 * r
                    off = 128 * j if j >= 0 else 0
                    st = st_pool.tile([128, QR], F32, tag="st")
                    nc.tensor.matmul(
                        st[:, off:QR],
                        kslice(kt),
                        qTs[r][:, off:QR],
                        start=True, stop=True)
                    e = e_pool.tile([128, QR], BF16, tag="e",
                                    name=f"e{r}_{kt}")
                    if j >= 1:
                        # dead region zeroed so l-tree adds are full-width
                        nc.vector.memset(e[:, 0:off], 0.0)
                    nc.scalar.activation(
                        e[:, off:QR], st[:, off:QR],
                        mybir.ActivationFunctionType.Exp,
                        scale=float(SCALE))
                    if j >= 0:
                        nc.vector.tensor_mul(
                            e[:, off:off + 128],
                            e[:, off:off + 128],
                            mask_sb[:])
                    es[kt] = e
                all_e[r] = es

            def emit_lsum(r):
                # bf16 binary tree on DVE, then one ones-matmul on PE
                lvl = list(all_e[r])
                while len(lvl) > 1:
                    nxt = []
                    for i in range(0, len(lvl) - 1, 2):
                        dst = ep_pool.tile([128, QR], BF16, tag="ep")
                        nc.vector.tensor_add(dst[:], lvl[i][:], lvl[i + 1][:])
                        nxt.append(dst)
                    if len(lvl) % 2:
                        nxt.append(lvl[-1])
                    lvl = nxt
                l_ps = acc_pool.tile([1, QR], F32, tag="lacc")
                nc.tensor.matmul(l_ps[:], ones_sb[:], lvl[0][:],
                                 start=True, stop=True)
                ls = osmall_pool.tile([1, QR], F32, tag=f"l{r}",
                                      name=f"l{r}")
                nc.scalar.copy(ls[:], l_ps[:])
                nc.sync.dma_start(l_d.ap()[:, QR * r:QR * (r + 1)], ls[:])

            def emit_pv(r):
                nkt = 4 * r + 4
                es = all_e.pop(r)
                o_ps = acc_pool.tile([128, QR], F32, tag="outT")
                for i, kt in enumerate(range(nkt)):
                    j = kt - 4 * r
                    off = 128 * j if j >= 0 else 0
                    nc.tensor.matmul(
                        o_ps[:, off:QR],
                        vslice(kt),
                        es[kt][:, off:QR],
                        start=(i == 0), stop=(i == nkt - 1),
                        skip_group_check=True)
                ot = osmall_pool.tile([128, QR], F32, tag=f"outT{r}",
                                      name=f"ot{r}")
                nc.vector.tensor_copy(ot[:], o_ps[:])
                nc.sync.dma_start(out_d.ap()[:, QR * r:QR * (r + 1)], ot[:])

            # merged schedule: each x segment unlocks attention work
            emit_qkv(0)
            emit_vtr(0)
            emit_st(0)
            emit_qkv(1)
            emit_vtr(1)
            emit_st(1)
            emit_lsum(0)
            emit_pv(0)
            emit_qkv(2)
            emit_vtr(2)
            emit_st(2)
            emit_lsum(1)
            emit_pv(1)
            emit_qkv(3)
            emit_vtr(3)
            emit_st(3)
            emit_lsum(2)
            emit_pv(2)
            emit_lsum(3)
            emit_pv(3)

    nc.compile()
    return nc


_PROGRAM = None


def _get_program():
    global _PROGRAM
    if _PROGRAM is None:
        _PROGRAM = _build_program()
    return _PROGRAM


import ml_dtypes

BF16_NP = ml_dtypes.bfloat16


def _host_inputs(x, Wq, Wk, Wv):
    x = np.asarray(x, dtype=np.float32)
    Wq = np.asarray(Wq, dtype=np.float32)
    Wk = np.asarray(Wk, dtype=np.float32)
    Wv = np.asarray(Wv, dtype=np.float32)

    p = np.arange(128)[:, None]
    f = np.arange(128)[None, :]
    mask = (f >= p).astype(BF16_NP)
    ones = np.ones((128, 1), dtype=BF16_NP)
    wstack = np.stack([Wq, Wk, Wv])  # [3, C, H]
    wstack = wstack.reshape(3, NCC, 128, H).transpose(2, 0, 1, 3)
    wstack = np.ascontiguousarray(wstack.reshape(128, 3 * NCC * H)
                                  .astype(BF16_NP))

    in_maps = []
    for b in range(NCORES):
        xb = x[b].T.astype(BF16_NP)                       # [C, T]
        xb = xb.reshape(NCC, 128, NQR, QR).transpose(2, 1, 0, 3)
        in_maps.append({
            "x": np.ascontiguousarray(xb),
            "w": wstack, "mask": mask, "ones": ones,
        })
    return in_maps


def run(x, Wq, Wk, Wv, trace=False, **kwargs):
    nc = _get_program()
    in_maps = _host_inputs(x, Wq, Wk, Wv)
    res = run_bass_kernel_spmd(nc, in_maps, core_ids=list(range(NCORES)),
                               trace=trace, **kwargs)
    outs = []
    for b in range(NCORES):
        oT = res.results[b]["out"].astype(np.float32)     # [H, T]
        l = res.results[b]["l"].astype(np.float32)        # [1, T]
        outs.append((oT / l).T)
    return np.stack(outs, axis=0).astype(np.float32), res


def kernel(x, Wq, Wk, Wv):
    out, _ = run(x, Wq, Wk, Wv)
    return out
